# revision 18
# baseline (speedup 1.0000x reference)
"""Trainium2 Bass kernel: per-sample bone-length MSE loss.

loss[s] = sum_k ( ||jo[s,a_k]-jo[s,b_k]|| - ||jg[s,a_k]-jg[s,b_k]|| )^2

Strategy: pure data parallel over 8 NeuronCores (batch split). Natural
layout: samples on the 128 SBUF partitions, T samples per partition per
tile. The skeleton gather is specialized at build time into a few grouped
strided tensor_sub instructions (DVE); squares and sqrt run on ACT.
"""

import numpy as np
from contextlib import ExitStack

import concourse.bass as bass
import concourse.mybir as mybir
import concourse.tile as tile
from concourse import bacc
from concourse.bass_utils import run_bass_kernel_spmd

B, J, K = 1048576, 21, 20
NCORES = 8
BS = B // NCORES             # 131072 samples per core
P = 128                      # SBUF partitions
T = 64                       # samples per partition per tile
TILE_SAMPLES = P * T         # 8192
NTILES = BS // TILE_SAMPLES  # 16
W = 3 * J                    # 63 floats per sample

_HAND = [(0, 1), (1, 2), (2, 3), (3, 4),
         (0, 5), (5, 6), (6, 7), (7, 8),
         (0, 9), (9, 10), (10, 11), (11, 12),
         (0, 13), (13, 14), (14, 15), (15, 16),
         (0, 17), (17, 18), (18, 19), (19, 20)]


def plan_sub_groups(skel):
    """Cover all bones with as few strided 2D grid specs as possible.

    Spec covers bones k = k0 + i*dkO + j*dkI (i<m, j<n) with joint indices
    a = a0 + i*daO + j*daI and b likewise. Each spec becomes one DVE
    tensor_sub instruction.
    """
    bones = [(int(a), int(b)) for a, b in skel]
    if bones == _HAND:
        specs = [
            dict(k0=0, a0=0, b0=1, m=1, dkO=0, daO=0, dbO=0, n=4, dkI=1, daI=1, dbI=1),
            dict(k0=4, a0=0, b0=5, m=4, dkO=4, daO=0, dbO=4, n=1, dkI=0, daI=0, dbI=0),
            dict(k0=5, a0=5, b0=6, m=4, dkO=4, daO=4, dbO=4, n=3, dkI=1, daI=1, dbI=1),
        ]
    else:
        # greedy 1D runs
        runs = []
        i = 0
        nb = len(bones)
        while i < nb:
            if i + 1 < nb:
                da = bones[i + 1][0] - bones[i][0]
                db = bones[i + 1][1] - bones[i][1]
                n = 2
                while (i + n < nb
                       and bones[i + n][0] - bones[i + n - 1][0] == da
                       and bones[i + n][1] - bones[i + n - 1][1] == db):
                    n += 1
            else:
                da = db = 0
                n = 1
            if n == 1:
                da = db = 0
            runs.append(dict(k0=i, a0=bones[i][0], b0=bones[i][1],
                             n=n, dkI=1 if n > 1 else 0, daI=da, dbI=db))
            i += n
        # merge compatible runs into 2D grids (same inner shape, arithmetic offsets)
        specs = []
        used = [False] * len(runs)
        for ri, r in enumerate(runs):
            if used[ri]:
                continue
            grp = [r]
            used[ri] = True
            dk = da = db = None
            for rj in range(ri + 1, len(runs)):
                r2 = runs[rj]
                if (used[rj] or r2['n'] != r['n']
                        or r2['daI'] != r['daI'] or r2['dbI'] != r['dbI']):
                    continue
                last = grp[-1]
                step = (r2['k0'] - last['k0'], r2['a0'] - last['a0'], r2['b0'] - last['b0'])
                if dk is None:
                    dk, da, db = step
                    grp.append(r2)
                    used[rj] = True
                elif step == (dk, da, db):
                    grp.append(r2)
                    used[rj] = True
            if len(grp) == 1:
                specs.append(dict(k0=r['k0'], a0=r['a0'], b0=r['b0'],
                                  m=1, dkO=0, daO=0, dbO=0,
                                  n=r['n'], dkI=r['dkI'], daI=r['daI'], dbI=r['dbI']))
            else:
                specs.append(dict(k0=r['k0'], a0=r['a0'], b0=r['b0'],
                                  m=len(grp), dkO=dk, daO=da, dbO=db,
                                  n=r['n'], dkI=r['dkI'], daI=r['daI'], dbI=r['dbI']))
    # verify exact coverage
    covered = {}
    for s in specs:
        for i in range(s['m']):
            for j in range(s['n']):
                k = s['k0'] + i * s['dkO'] + j * s['dkI']
                covered[k] = (s['a0'] + i * s['daO'] + j * s['daI'],
                              s['b0'] + i * s['dbO'] + j * s['dbI'])
    ok = (len(covered) == len(bones)
          and all(covered.get(k) == bones[k] for k in range(len(bones))))
    if not ok:
        specs = [dict(k0=k, a0=a, b0=b, m=1, dkO=0, daO=0, dbO=0,
                      n=1, dkI=0, daI=0, dbI=0) for k, (a, b) in enumerate(bones)]
    return specs


def _ap(base, off, dims):
    """Custom strided AP on an SBUF tile: partition dim + given free dims."""
    return bass.AP(tensor=base.tensor, offset=base.offset + off,
                   ap=[list(base.ap[0])] + [list(d) for d in dims])


def build_program(skel, t=T, ntiles=NTILES, repeat=1, mode="full", xbufs=2):
    specs = plan_sub_groups(skel)
    nc = bacc.Bacc("TRN2", target_bir_lowering=False, debug=False,
                   num_devices=NCORES)
    f32 = mybir.dt.float32
    jo = nc.declare_dram_parameter("joint_out", [ntiles, P, t * W], f32, isOutput=False)
    jg = nc.declare_dram_parameter("joint_gt", [ntiles, P, t * W], f32, isOutput=False)
    lo = nc.declare_dram_parameter("loss", [ntiles, P, t], f32, isOutput=True)

    ST = 2 * t
    with tile.TileContext(nc) as tc:
        with ExitStack() as ctx:
            xpool = ctx.enter_context(tc.tile_pool(name="x", bufs=xbufs))
            dpool = ctx.enter_context(tc.tile_pool(name="d", bufs=2))
            lpool = ctx.enter_context(tc.tile_pool(name="l", bufs=2))
            opool = ctx.enter_context(tc.tile_pool(name="o", bufs=4))
            for _ in range(repeat):
                for i in range(ntiles):
                    X = xpool.tile([P, 2 * t * W], f32)
                    nc.sync.dma_start(X[:, 0:t * W], jo[i])
                    nc.sync.dma_start(X[:, t * W:2 * t * W], jg[i])

                    if mode == "dma":
                        LOSS = opool.tile([P, t], f32)
                        nc.vector.tensor_copy(LOSS[:], X[:, 0:t])
                        nc.vector.tensor_copy(LOSS[:], X[:, t * W:t * W + t])
                        nc.sync.dma_start(lo[i], LOSS[:])
                        continue
                    if mode in ("act", "act2"):
                        if mode == "act":
                            nc.scalar.square(X[:], X[:])
                            src = X
                        else:
                            D2 = dpool.tile([P, 2 * t * W], f32)
                            nc.scalar.square(D2[:], X[:])
                            src = D2
                        LOSS = opool.tile([P, t], f32)
                        nc.vector.tensor_copy(LOSS[:], src[:, 0:t])
                        nc.sync.dma_start(lo[i], LOSS[:])
                        continue

                    # D[s][t][k][c] = X[s][t][3*a_k+c] - X[s][t][3*b_k+c]
                    D = dpool.tile([P, 120 * t], f32)
                    for s in specs:
                        m, n = s['m'], s['n']
                        d_out = [[60, ST]]
                        d_a = [[W, ST]]
                        d_b = [[W, ST]]
                        if m > 1:
                            d_out.append([3 * s['dkO'], m])
                            d_a.append([3 * s['daO'], m])
                            d_b.append([3 * s['dbO'], m])
                        if n > 1 and s['dkI'] == 1 and s['daI'] == 1 and s['dbI'] == 1:
                            d_out.append([1, 3 * n])
                            d_a.append([1, 3 * n])
                            d_b.append([1, 3 * n])
                        elif n > 1:
                            d_out += [[3 * s['dkI'], n], [1, 3]]
                            d_a += [[3 * s['daI'], n], [1, 3]]
                            d_b += [[3 * s['dbI'], n], [1, 3]]
                        else:
                            d_out.append([1, 3])
                            d_a.append([1, 3])
                            d_b.append([1, 3])
                        nc.vector.tensor_sub(_ap(D[:], 3 * s['k0'], d_out),
                                             _ap(X[:], 3 * s['a0'], d_a),
                                             _ap(X[:], 3 * s['b0'], d_b))

                    # squared diffs (in place, ACT)
                    if mode != "dve":
                        nc.scalar.square(D[:], D[:])

                    # LSQ[s][t][k] = sum_c D[s][t][k][c]
                    LSQ = lpool.tile([P, 40 * t], f32)
                    nc.vector.tensor_add(_ap(LSQ[:], 0, [[20, ST], [1, 20]]),
                                         _ap(D[:], 0, [[60, ST], [3, 20]]),
                                         _ap(D[:], 1, [[60, ST], [3, 20]]))
                    nc.vector.tensor_add(_ap(LSQ[:], 0, [[20, ST], [1, 20]]),
                                         _ap(LSQ[:], 0, [[20, ST], [1, 20]]),
                                         _ap(D[:], 2, [[60, ST], [3, 20]]))

                    # bone lengths (in place, ACT)
                    if mode != "dve":
                        nc.scalar.sqrt(LSQ[:], LSQ[:])

                    # DL[t][k] = L_out - L_gt ; then squared (ACT)
                    DL = lpool.tile([P, 20 * t], f32)
                    nc.vector.tensor_sub(DL[:], LSQ[:, 0:20 * t], LSQ[:, 20 * t:40 * t])
                    if mode != "dve":
                        nc.scalar.square(DL[:], DL[:])

                    # loss[t] = sum_k DL[t][k]
                    LOSS = opool.tile([P, t], f32)
                    nc.vector.reduce_sum(LOSS[:],
                                         _ap(DL[:], 0, [[20, t], [1, 20]]),
                                         axis=mybir.AxisListType.X)
                    nc.sync.dma_start(lo[i], LOSS[:])
    nc.compile()
    return nc


# ---------------------------------------------------------------------------
# v2: transposed per-partition layout (samples innermost), fp16 on-chip.
#
# DRAM layout (repacked on host): [ntiles, P, 63, S] fp32 — sample index
# s = ((tile*P)+p)*S + st. All on-chip tensors put st innermost (step 1,
# S-aligned starts) so fp16 DVE ops hit the 2x_1P perf mode.
# ---------------------------------------------------------------------------

S2 = 128                      # samples per partition per tile (v2)
NTILES2 = BS // (P * S2)      # 8


def build_program_v2(skel, s=S2, ntiles=NTILES2, repeat=1, in_dtype="float32",
                     xbufs=2, dbufs=2, lbufs=2, tree_reduce=False, dl2_act=False,
                     fuse_in=False):
    specs = plan_sub_groups(skel)
    nc = bacc.Bacc("TRN2", target_bir_lowering=False, debug=False,
                   num_devices=NCORES)
    f32 = mybir.dt.float32
    f16 = mybir.dt.float16
    in_dt = f32 if in_dtype == "float32" else f16
    if fuse_in:
        jj = nc.declare_dram_parameter("joints", [ntiles, P, 2 * W * s], in_dt,
                                       isOutput=False)
    else:
        jo = nc.declare_dram_parameter("joint_out", [ntiles, P, W * s], in_dt, isOutput=False)
        jg = nc.declare_dram_parameter("joint_gt", [ntiles, P, W * s], in_dt, isOutput=False)
    lo = nc.declare_dram_parameter("loss", [ntiles, P, s], f32, isOutput=True)
    dma_in = nc.gpsimd if in_dtype == "float32" else nc.sync

    with tile.TileContext(nc) as tc:
        with ExitStack() as ctx:
            xpool = ctx.enter_context(tc.tile_pool(name="x", bufs=xbufs))
            dpool = ctx.enter_context(tc.tile_pool(name="d", bufs=dbufs))
            lpool = ctx.enter_context(tc.tile_pool(name="l", bufs=lbufs))
            opool = ctx.enter_context(tc.tile_pool(name="o", bufs=4))
            for _ in range(repeat):
                for i in range(ntiles):
                    # X[s][j3c][st] fp16 (cast during DMA when DRAM is fp32)
                    X = xpool.tile([P, 2 * W * s], f16)
                    if fuse_in:
                        dma_in.dma_start(X[:], jj[i])
                    else:
                        dma_in.dma_start(X[:, 0:W * s], jo[i])
                        dma_in.dma_start(X[:, W * s:2 * W * s], jg[i])

                    # D[set][k][c][st] = X[.,a_k,c,st] - X[.,b_k,c,st]
                    D = dpool.tile([P, 120 * s], f16)
                    for g in specs:
                        m, n = g['m'], g['n']
                        d_out = [[60 * s, 2]]
                        d_a = [[W * s, 2]]
                        d_b = [[W * s, 2]]
                        if m > 1:
                            d_out.append([3 * s * g['dkO'], m])
                            d_a.append([3 * s * g['daO'], m])
                            d_b.append([3 * s * g['dbO'], m])
                        if n > 1:
                            d_out.append([3 * s * g['dkI'], n])
                            d_a.append([3 * s * g['daI'], n])
                            d_b.append([3 * s * g['dbI'], n])
                        d_out.append([1, 3 * s])
                        d_a.append([1, 3 * s])
                        d_b.append([1, 3 * s])
                        nc.vector.tensor_sub(_ap(D[:], 3 * s * g['k0'], d_out),
                                             _ap(X[:], 3 * s * g['a0'], d_a),
                                             _ap(X[:], 3 * s * g['b0'], d_b))

                    # squared diffs (in place, ACT)
                    nc.scalar.square(D[:], D[:])

                    # LSQ[set][k][st] = sum_c D[set][k][c][st]
                    LSQ = lpool.tile([P, 40 * s], f16)
                    cplane = lambda c: _ap(D[:], c * s,
                                           [[60 * s, 2], [3 * s, 20], [1, s]])
                    nc.vector.tensor_add(LSQ[:], cplane(0), cplane(1))
                    nc.vector.tensor_add(LSQ[:], LSQ[:], cplane(2))

                    # bone lengths (in place, ACT)
                    nc.scalar.sqrt(LSQ[:], LSQ[:])

                    # DL[k][st] = L_out - L_gt, then squared
                    DL = lpool.tile([P, 20 * s], f16)
                    nc.vector.tensor_sub(DL[:], LSQ[:, 0:20 * s], LSQ[:, 20 * s:40 * s])
                    if dl2_act:
                        nc.scalar.square(DL[:], DL[:])
                    else:
                        nc.vector.tensor_mul(DL[:], DL[:], DL[:])

                    # loss[st] = sum_k DL[k][st]  (fp32 accumulate)
                    LOSS = opool.tile([P, s], f32)
                    if tree_reduce:
                        # fp16 2x-mode add tree over the 20 bones:
                        # 20 -> 10 -> 5 -> (4 -> 2 -> 1) + odd plane
                        nc.vector.tensor_add(DL[:, 0:10 * s],
                                             DL[:, 0:10 * s], DL[:, 10 * s:20 * s])
                        nc.vector.tensor_add(DL[:, 0:5 * s],
                                             DL[:, 0:5 * s], DL[:, 5 * s:10 * s])
                        nc.vector.tensor_add(DL[:, 0:2 * s],
                                             DL[:, 0:2 * s], DL[:, 2 * s:4 * s])
                        nc.vector.tensor_add(DL[:, 0:s],
                                             DL[:, 0:s], DL[:, s:2 * s])
                        nc.vector.tensor_add(LOSS[:],
                                             DL[:, 0:s], DL[:, 4 * s:5 * s])
                    else:
                        nc.vector.reduce_sum(LOSS[:],
                                             _ap(DL[:], 0, [[1, s], [s, 20]]),
                                             axis=mybir.AxisListType.X)
                    nc.sync.dma_start(lo[i], LOSS[:])
    nc.compile()
    return nc


def _repack_v2(arr, ntiles, s, dtype=np.float32):
    """[BS per-core slice, W] fp32 -> [ntiles, P, W, s] with st innermost."""
    out = np.ascontiguousarray(
        arr.reshape(ntiles, P, s, W).transpose(0, 1, 3, 2)).astype(dtype, copy=False)
    return out.reshape(ntiles, P, W * s)


_cache = {}


def _get_program(skel):
    key = skel.tobytes()
    if key not in _cache:
        _cache[key] = build_program_v2(skel, in_dtype="float16", tree_reduce=True,
                                       fuse_in=True, dbufs=3, lbufs=3)
    return _cache[key]


def _fused_input(jo_slice, jg_slice, ntiles=NTILES2, s=S2):
    """Stack both joint sets into one [ntiles, P, 2*W*s] fp16 array."""
    a = _repack_v2(jo_slice, ntiles, s, np.float16).reshape(ntiles, P, W, s)
    b = _repack_v2(jg_slice, ntiles, s, np.float16).reshape(ntiles, P, W, s)
    return np.concatenate([a, b], axis=2).reshape(ntiles, P, 2 * W * s)


def kernel(joint_out, joint_gt, skeleton):
    skel = np.asarray(skeleton, dtype=np.int32)
    nc = _get_program(skel)
    jo = np.ascontiguousarray(np.asarray(joint_out, dtype=np.float32)).reshape(B, W)
    jg = np.ascontiguousarray(np.asarray(joint_gt, dtype=np.float32)).reshape(B, W)
    in_maps = []
    for c in range(NCORES):
        sl = slice(c * BS, (c + 1) * BS)
        in_maps.append({"joints": _fused_input(jo[sl], jg[sl])})
    res = run_bass_kernel_spmd(nc, in_maps, core_ids=list(range(NCORES)))
    out = np.empty(B, np.float32)
    for c in range(NCORES):
        out[c * BS:(c + 1) * BS] = res.results[c]["loss"].reshape(BS)
    return out


# revision 19
# speedup vs baseline: 1.0766x; 1.0766x over previous
"""Trainium2 Bass kernel: per-sample bone-length MSE loss.

loss[s] = sum_k ( ||jo[s,a_k]-jo[s,b_k]|| - ||jg[s,a_k]-jg[s,b_k]|| )^2

Strategy: pure data parallel over 8 NeuronCores (batch split). The active
path is build_program_v2: inputs are repacked on the host into a fused
[ntiles, 128, 2*63*S] fp16 array with SAMPLES INNERMOST per partition
(S=128 samples/partition/tile), so every strided DVE operand has a
unit-stride, 4-byte-aligned innermost run and fp16 tensor_tensor ops hit
the 2x_1P perf mode. The skeleton gather is specialized at build time
into a few grouped strided tensor_sub instructions (DVE); the big square
and sqrt run on ACT (fp16 is 2x there); the bone-axis reduction is an
fp16 add tree (beats 1x-mode reduce_sum). Measured ~130-140us/core vs a
~195us fp32 memory roofline; DVE-bound at the 2-read-port floor.

(build_program / v1 below is the earlier fp32 natural-layout version,
kept for reference and benchmark probes; kernel() uses v2 only.)
"""

import numpy as np
from contextlib import ExitStack

import concourse.bass as bass
import concourse.mybir as mybir
import concourse.tile as tile
from concourse import bacc
from concourse.bass_utils import run_bass_kernel_spmd

B, J, K = 1048576, 21, 20
NCORES = 8
BS = B // NCORES             # 131072 samples per core
P = 128                      # SBUF partitions
T = 64                       # samples per partition per tile
TILE_SAMPLES = P * T         # 8192
NTILES = BS // TILE_SAMPLES  # 16
W = 3 * J                    # 63 floats per sample

_HAND = [(0, 1), (1, 2), (2, 3), (3, 4),
         (0, 5), (5, 6), (6, 7), (7, 8),
         (0, 9), (9, 10), (10, 11), (11, 12),
         (0, 13), (13, 14), (14, 15), (15, 16),
         (0, 17), (17, 18), (18, 19), (19, 20)]


def plan_sub_groups(skel):
    """Cover all bones with as few strided 2D grid specs as possible.

    Spec covers bones k = k0 + i*dkO + j*dkI (i<m, j<n) with joint indices
    a = a0 + i*daO + j*daI and b likewise. Each spec becomes one DVE
    tensor_sub instruction.
    """
    bones = [(int(a), int(b)) for a, b in skel]
    if bones == _HAND:
        specs = [
            dict(k0=0, a0=0, b0=1, m=1, dkO=0, daO=0, dbO=0, n=4, dkI=1, daI=1, dbI=1),
            dict(k0=4, a0=0, b0=5, m=4, dkO=4, daO=0, dbO=4, n=1, dkI=0, daI=0, dbI=0),
            dict(k0=5, a0=5, b0=6, m=4, dkO=4, daO=4, dbO=4, n=3, dkI=1, daI=1, dbI=1),
        ]
    else:
        # greedy 1D runs
        runs = []
        i = 0
        nb = len(bones)
        while i < nb:
            if i + 1 < nb:
                da = bones[i + 1][0] - bones[i][0]
                db = bones[i + 1][1] - bones[i][1]
                n = 2
                while (i + n < nb
                       and bones[i + n][0] - bones[i + n - 1][0] == da
                       and bones[i + n][1] - bones[i + n - 1][1] == db):
                    n += 1
            else:
                da = db = 0
                n = 1
            if n == 1:
                da = db = 0
            runs.append(dict(k0=i, a0=bones[i][0], b0=bones[i][1],
                             n=n, dkI=1 if n > 1 else 0, daI=da, dbI=db))
            i += n
        # merge compatible runs into 2D grids (same inner shape, arithmetic offsets)
        specs = []
        used = [False] * len(runs)
        for ri, r in enumerate(runs):
            if used[ri]:
                continue
            grp = [r]
            used[ri] = True
            dk = da = db = None
            for rj in range(ri + 1, len(runs)):
                r2 = runs[rj]
                if (used[rj] or r2['n'] != r['n']
                        or r2['daI'] != r['daI'] or r2['dbI'] != r['dbI']):
                    continue
                last = grp[-1]
                step = (r2['k0'] - last['k0'], r2['a0'] - last['a0'], r2['b0'] - last['b0'])
                if dk is None:
                    dk, da, db = step
                    grp.append(r2)
                    used[rj] = True
                elif step == (dk, da, db):
                    grp.append(r2)
                    used[rj] = True
            if len(grp) == 1:
                specs.append(dict(k0=r['k0'], a0=r['a0'], b0=r['b0'],
                                  m=1, dkO=0, daO=0, dbO=0,
                                  n=r['n'], dkI=r['dkI'], daI=r['daI'], dbI=r['dbI']))
            else:
                specs.append(dict(k0=r['k0'], a0=r['a0'], b0=r['b0'],
                                  m=len(grp), dkO=dk, daO=da, dbO=db,
                                  n=r['n'], dkI=r['dkI'], daI=r['daI'], dbI=r['dbI']))
    # verify exact coverage
    covered = {}
    for s in specs:
        for i in range(s['m']):
            for j in range(s['n']):
                k = s['k0'] + i * s['dkO'] + j * s['dkI']
                covered[k] = (s['a0'] + i * s['daO'] + j * s['daI'],
                              s['b0'] + i * s['dbO'] + j * s['dbI'])
    ok = (len(covered) == len(bones)
          and all(covered.get(k) == bones[k] for k in range(len(bones))))
    if not ok:
        specs = [dict(k0=k, a0=a, b0=b, m=1, dkO=0, daO=0, dbO=0,
                      n=1, dkI=0, daI=0, dbI=0) for k, (a, b) in enumerate(bones)]
    return specs


def _ap(base, off, dims):
    """Custom strided AP on an SBUF tile: partition dim + given free dims."""
    return bass.AP(tensor=base.tensor, offset=base.offset + off,
                   ap=[list(base.ap[0])] + [list(d) for d in dims])


def build_program(skel, t=T, ntiles=NTILES, repeat=1, mode="full", xbufs=2):
    specs = plan_sub_groups(skel)
    nc = bacc.Bacc("TRN2", target_bir_lowering=False, debug=False,
                   num_devices=NCORES)
    f32 = mybir.dt.float32
    jo = nc.declare_dram_parameter("joint_out", [ntiles, P, t * W], f32, isOutput=False)
    jg = nc.declare_dram_parameter("joint_gt", [ntiles, P, t * W], f32, isOutput=False)
    lo = nc.declare_dram_parameter("loss", [ntiles, P, t], f32, isOutput=True)

    ST = 2 * t
    with tile.TileContext(nc) as tc:
        with ExitStack() as ctx:
            xpool = ctx.enter_context(tc.tile_pool(name="x", bufs=xbufs))
            dpool = ctx.enter_context(tc.tile_pool(name="d", bufs=2))
            lpool = ctx.enter_context(tc.tile_pool(name="l", bufs=2))
            opool = ctx.enter_context(tc.tile_pool(name="o", bufs=4))
            for _ in range(repeat):
                for i in range(ntiles):
                    X = xpool.tile([P, 2 * t * W], f32)
                    nc.sync.dma_start(X[:, 0:t * W], jo[i])
                    nc.sync.dma_start(X[:, t * W:2 * t * W], jg[i])

                    if mode == "dma":
                        LOSS = opool.tile([P, t], f32)
                        nc.vector.tensor_copy(LOSS[:], X[:, 0:t])
                        nc.vector.tensor_copy(LOSS[:], X[:, t * W:t * W + t])
                        nc.sync.dma_start(lo[i], LOSS[:])
                        continue
                    if mode in ("act", "act2"):
                        if mode == "act":
                            nc.scalar.square(X[:], X[:])
                            src = X
                        else:
                            D2 = dpool.tile([P, 2 * t * W], f32)
                            nc.scalar.square(D2[:], X[:])
                            src = D2
                        LOSS = opool.tile([P, t], f32)
                        nc.vector.tensor_copy(LOSS[:], src[:, 0:t])
                        nc.sync.dma_start(lo[i], LOSS[:])
                        continue

                    # D[s][t][k][c] = X[s][t][3*a_k+c] - X[s][t][3*b_k+c]
                    D = dpool.tile([P, 120 * t], f32)
                    for s in specs:
                        m, n = s['m'], s['n']
                        d_out = [[60, ST]]
                        d_a = [[W, ST]]
                        d_b = [[W, ST]]
                        if m > 1:
                            d_out.append([3 * s['dkO'], m])
                            d_a.append([3 * s['daO'], m])
                            d_b.append([3 * s['dbO'], m])
                        if n > 1 and s['dkI'] == 1 and s['daI'] == 1 and s['dbI'] == 1:
                            d_out.append([1, 3 * n])
                            d_a.append([1, 3 * n])
                            d_b.append([1, 3 * n])
                        elif n > 1:
                            d_out += [[3 * s['dkI'], n], [1, 3]]
                            d_a += [[3 * s['daI'], n], [1, 3]]
                            d_b += [[3 * s['dbI'], n], [1, 3]]
                        else:
                            d_out.append([1, 3])
                            d_a.append([1, 3])
                            d_b.append([1, 3])
                        nc.vector.tensor_sub(_ap(D[:], 3 * s['k0'], d_out),
                                             _ap(X[:], 3 * s['a0'], d_a),
                                             _ap(X[:], 3 * s['b0'], d_b))

                    # squared diffs (in place, ACT)
                    if mode != "dve":
                        nc.scalar.square(D[:], D[:])

                    # LSQ[s][t][k] = sum_c D[s][t][k][c]
                    LSQ = lpool.tile([P, 40 * t], f32)
                    nc.vector.tensor_add(_ap(LSQ[:], 0, [[20, ST], [1, 20]]),
                                         _ap(D[:], 0, [[60, ST], [3, 20]]),
                                         _ap(D[:], 1, [[60, ST], [3, 20]]))
                    nc.vector.tensor_add(_ap(LSQ[:], 0, [[20, ST], [1, 20]]),
                                         _ap(LSQ[:], 0, [[20, ST], [1, 20]]),
                                         _ap(D[:], 2, [[60, ST], [3, 20]]))

                    # bone lengths (in place, ACT)
                    if mode != "dve":
                        nc.scalar.sqrt(LSQ[:], LSQ[:])

                    # DL[t][k] = L_out - L_gt ; then squared (ACT)
                    DL = lpool.tile([P, 20 * t], f32)
                    nc.vector.tensor_sub(DL[:], LSQ[:, 0:20 * t], LSQ[:, 20 * t:40 * t])
                    if mode != "dve":
                        nc.scalar.square(DL[:], DL[:])

                    # loss[t] = sum_k DL[t][k]
                    LOSS = opool.tile([P, t], f32)
                    nc.vector.reduce_sum(LOSS[:],
                                         _ap(DL[:], 0, [[20, t], [1, 20]]),
                                         axis=mybir.AxisListType.X)
                    nc.sync.dma_start(lo[i], LOSS[:])
    nc.compile()
    return nc


# ---------------------------------------------------------------------------
# v2: transposed per-partition layout (samples innermost), fp16 on-chip.
#
# DRAM layout (repacked on host): [ntiles, P, 63, S] fp32 — sample index
# s = ((tile*P)+p)*S + st. All on-chip tensors put st innermost (step 1,
# S-aligned starts) so fp16 DVE ops hit the 2x_1P perf mode.
# ---------------------------------------------------------------------------

S2 = 128                      # samples per partition per tile (v2)
NTILES2 = BS // (P * S2)      # 8


def build_program_v2(skel, s=S2, ntiles=NTILES2, repeat=1, in_dtype="float32",
                     xbufs=2, dbufs=2, lbufs=2, tree_reduce=False, dl2_act=False,
                     fuse_in=False):
    specs = plan_sub_groups(skel)
    nc = bacc.Bacc("TRN2", target_bir_lowering=False, debug=False,
                   num_devices=NCORES)
    f32 = mybir.dt.float32
    f16 = mybir.dt.float16
    in_dt = f32 if in_dtype == "float32" else f16
    if fuse_in:
        jj = nc.declare_dram_parameter("joints", [ntiles, P, 2 * W * s], in_dt,
                                       isOutput=False)
    else:
        jo = nc.declare_dram_parameter("joint_out", [ntiles, P, W * s], in_dt, isOutput=False)
        jg = nc.declare_dram_parameter("joint_gt", [ntiles, P, W * s], in_dt, isOutput=False)
    lo = nc.declare_dram_parameter("loss", [ntiles, P, s], f32, isOutput=True)
    dma_in = nc.gpsimd if in_dtype == "float32" else nc.sync

    with tile.TileContext(nc) as tc:
        with ExitStack() as ctx:
            xpool = ctx.enter_context(tc.tile_pool(name="x", bufs=xbufs))
            dpool = ctx.enter_context(tc.tile_pool(name="d", bufs=dbufs))
            lpool = ctx.enter_context(tc.tile_pool(name="l", bufs=lbufs))
            opool = ctx.enter_context(tc.tile_pool(name="o", bufs=4))
            for _ in range(repeat):
                for i in range(ntiles):
                    # X[s][j3c][st] fp16 (cast during DMA when DRAM is fp32)
                    X = xpool.tile([P, 2 * W * s], f16)
                    if fuse_in:
                        dma_in.dma_start(X[:], jj[i])
                    else:
                        dma_in.dma_start(X[:, 0:W * s], jo[i])
                        dma_in.dma_start(X[:, W * s:2 * W * s], jg[i])

                    # D[set][k][c][st] = X[.,a_k,c,st] - X[.,b_k,c,st]
                    D = dpool.tile([P, 120 * s], f16)
                    for g in specs:
                        m, n = g['m'], g['n']
                        d_out = [[60 * s, 2]]
                        d_a = [[W * s, 2]]
                        d_b = [[W * s, 2]]
                        if m > 1:
                            d_out.append([3 * s * g['dkO'], m])
                            d_a.append([3 * s * g['daO'], m])
                            d_b.append([3 * s * g['dbO'], m])
                        if n > 1:
                            d_out.append([3 * s * g['dkI'], n])
                            d_a.append([3 * s * g['daI'], n])
                            d_b.append([3 * s * g['dbI'], n])
                        d_out.append([1, 3 * s])
                        d_a.append([1, 3 * s])
                        d_b.append([1, 3 * s])
                        nc.vector.tensor_sub(_ap(D[:], 3 * s * g['k0'], d_out),
                                             _ap(X[:], 3 * s * g['a0'], d_a),
                                             _ap(X[:], 3 * s * g['b0'], d_b))

                    # squared diffs (in place, ACT)
                    nc.scalar.square(D[:], D[:])

                    # LSQ[set][k][st] = sum_c D[set][k][c][st]
                    LSQ = lpool.tile([P, 40 * s], f16)
                    cplane = lambda c: _ap(D[:], c * s,
                                           [[60 * s, 2], [3 * s, 20], [1, s]])
                    nc.vector.tensor_add(LSQ[:], cplane(0), cplane(1))
                    nc.vector.tensor_add(LSQ[:], LSQ[:], cplane(2))

                    # bone lengths (in place, ACT)
                    nc.scalar.sqrt(LSQ[:], LSQ[:])

                    # DL[k][st] = L_out - L_gt, then squared
                    DL = lpool.tile([P, 20 * s], f16)
                    nc.vector.tensor_sub(DL[:], LSQ[:, 0:20 * s], LSQ[:, 20 * s:40 * s])
                    if dl2_act:
                        nc.scalar.square(DL[:], DL[:])
                    else:
                        nc.vector.tensor_mul(DL[:], DL[:], DL[:])

                    # loss[st] = sum_k DL[k][st]  (fp32 accumulate)
                    LOSS = opool.tile([P, s], f32)
                    if tree_reduce:
                        # fp16 2x-mode add tree over the 20 bones:
                        # 20 -> 10 -> 5 -> (4 -> 2 -> 1) + odd plane
                        nc.vector.tensor_add(DL[:, 0:10 * s],
                                             DL[:, 0:10 * s], DL[:, 10 * s:20 * s])
                        nc.vector.tensor_add(DL[:, 0:5 * s],
                                             DL[:, 0:5 * s], DL[:, 5 * s:10 * s])
                        nc.vector.tensor_add(DL[:, 0:2 * s],
                                             DL[:, 0:2 * s], DL[:, 2 * s:4 * s])
                        nc.vector.tensor_add(DL[:, 0:s],
                                             DL[:, 0:s], DL[:, s:2 * s])
                        nc.vector.tensor_add(LOSS[:],
                                             DL[:, 0:s], DL[:, 4 * s:5 * s])
                    else:
                        nc.vector.reduce_sum(LOSS[:],
                                             _ap(DL[:], 0, [[1, s], [s, 20]]),
                                             axis=mybir.AxisListType.X)
                    nc.sync.dma_start(lo[i], LOSS[:])
    nc.compile()
    return nc


def _repack_v2(arr, ntiles, s, dtype=np.float32):
    """[BS per-core slice, W] fp32 -> [ntiles, P, W, s] with st innermost."""
    out = np.ascontiguousarray(
        arr.reshape(ntiles, P, s, W).transpose(0, 1, 3, 2)).astype(dtype, copy=False)
    return out.reshape(ntiles, P, W * s)


_cache = {}


def _get_program(skel):
    key = skel.tobytes()
    if key not in _cache:
        _cache[key] = build_program_v2(skel, in_dtype="float16", tree_reduce=True,
                                       fuse_in=True, dbufs=3, lbufs=3)
    return _cache[key]


def _fused_input(jo_slice, jg_slice, ntiles=NTILES2, s=S2):
    """Stack both joint sets into one [ntiles, P, 2*W*s] fp16 array."""
    a = _repack_v2(jo_slice, ntiles, s, np.float16).reshape(ntiles, P, W, s)
    b = _repack_v2(jg_slice, ntiles, s, np.float16).reshape(ntiles, P, W, s)
    return np.concatenate([a, b], axis=2).reshape(ntiles, P, 2 * W * s)


def kernel(joint_out, joint_gt, skeleton):
    skel = np.asarray(skeleton, dtype=np.int32)
    nc = _get_program(skel)
    jo = np.ascontiguousarray(np.asarray(joint_out, dtype=np.float32)).reshape(B, W)
    jg = np.ascontiguousarray(np.asarray(joint_gt, dtype=np.float32)).reshape(B, W)
    in_maps = []
    for c in range(NCORES):
        sl = slice(c * BS, (c + 1) * BS)
        in_maps.append({"joints": _fused_input(jo[sl], jg[sl])})
    res = run_bass_kernel_spmd(nc, in_maps, core_ids=list(range(NCORES)))
    out = np.empty(B, np.float32)
    for c in range(NCORES):
        out[c * BS:(c + 1) * BS] = res.results[c]["loss"].reshape(BS)
    return out
